# revision 2
# baseline (speedup 1.0000x reference)
"""Trainium2 Bass kernel for nn_Block_9457517985872 (dense transformer block,
linear attention) — v2: bf16 matmuls, SBUF-resident intermediates.

Token-sharded across 8 NeuronCores: core c handles batch c//2, sequence half
c%2 (2048 tokens). Only cross-core communication is a pairwise AllReduce of
the per-head (kv, ksum) statistics [2,128,4,65] f32.

Self-contained: hardcodes all shapes from the problem spec.
"""
import numpy as np
import ml_dtypes
from contextlib import ExitStack

import concourse.bass as bass
import concourse.tile as tile
from concourse import bacc, mybir
from concourse.bass_utils import run_bass_kernel_spmd
from concourse.masks import make_identity

F32 = mybir.dt.float32
BF16 = mybir.dt.bfloat16
AF = mybir.ActivationFunctionType
ALU = mybir.AluOpType

B, N, C = 4, 4096, 1024
H, D = 16, 64
HID = 4096
TOK = 2048          # tokens per core
NT = TOK // 128     # 16 token tiles
NG = TOK // 512     # 4 token groups
EPS_LN = 1e-5
EPS_ATTN = 1e-6

_BUILD_CACHE = {}


def _build(flags, no_cc=False, cc_copy=False):
    """flags: (has_bk, has_bv, has_bproj, has_bfc2).
    no_cc: single-device build (for sim). cc_copy: 8-device build but the
    AllReduce replaced by a local copy (comm-setup cost probe)."""
    has_bk, has_bv, has_bproj, has_bfc2 = flags
    nc = bacc.Bacc("TRN2", target_bir_lowering=False, debug=False,
                   num_devices=1 if no_cc else 8)
    no_cc = no_cc or cc_copy

    xs = nc.dram_tensor("xs", [TOK, C], BF16, kind="ExternalInput")
    wq = nc.dram_tensor("wq", [128, 8, C], BF16, kind="ExternalInput")
    wkv = nc.dram_tensor("wkv", [128, 8, 2 * C], BF16, kind="ExternalInput")
    wp = nc.dram_tensor("wp", [128, 8, C], BF16, kind="ExternalInput")
    w1 = nc.dram_tensor("w1", [32, 128, 8, 128], BF16, kind="ExternalInput")
    w2 = nc.dram_tensor("w2", [128, 32, C], BF16, kind="ExternalInput")
    bq = nc.dram_tensor("bq", [C], F32, kind="ExternalInput")
    bk = nc.dram_tensor("bk", [C], F32, kind="ExternalInput")
    bv = nc.dram_tensor("bv", [C], F32, kind="ExternalInput")
    bg = nc.dram_tensor("bg", [HID], F32, kind="ExternalInput")
    bp = nc.dram_tensor("bp", [C], F32, kind="ExternalInput")
    b2o = nc.dram_tensor("b2o", [C], F32, kind="ExternalInput")
    out = nc.dram_tensor("out", [TOK, C], F32, kind="ExternalOutput")

    xs_v = xs.ap().rearrange("(t p) c -> t p c", p=128)     # [16,128,1024]
    out_v = out.ap().rearrange("(t p) c -> t p c", p=128)

    with tile.TileContext(nc) as tc, ExitStack() as ctx:
        const = ctx.enter_context(tc.tile_pool(name="const", bufs=1))
        dram = ctx.enter_context(tc.tile_pool(name="dram", bufs=1, space="DRAM"))
        statp = ctx.enter_context(tc.tile_pool(name="stat", bufs=4))

        ident = const.tile([128, 128], BF16)
        make_identity(nc, ident[:])
        eps_ln_t = const.tile([128, 1], F32)
        nc.vector.memset(eps_ln_t[:], EPS_LN)
        bq_sb = const.tile([128, 8], F32)
        nc.sync.dma_start(out=bq_sb[:], in_=bq.ap().rearrange("(oc p) -> p oc", p=128))
        bg_sb = const.tile([128, 32], F32)
        nc.sync.dma_start(out=bg_sb[:], in_=bg.ap().rearrange("(hd p) -> p hd", p=128))
        if has_bk:
            bk_bc = const.tile([128, C], F32)
            nc.sync.dma_start(out=bk_bc[:], in_=bass.AP(
                tensor=bk.ap().tensor, offset=0, ap=[[0, 128], [1, C]]))
        if has_bproj:
            bp_bc = const.tile([128, C], F32)
            nc.sync.dma_start(out=bp_bc[:], in_=bass.AP(
                tensor=bp.ap().tensor, offset=0, ap=[[0, 128], [1, C]]))
        if has_bfc2:
            b2_bc = const.tile([128, C], F32)
            nc.sync.dma_start(out=b2_bc[:], in_=bass.AP(
                tensor=b2o.ap().tensor, offset=0, ap=[[0, 128], [1, C]]))

        # LN1 stats (persist through phase 1)
        mvall = const.tile([128, NT, 2], F32)
        sdall = const.tile([128, NT], F32)
        rstdall = const.tile([128, NT], F32)
        nmrall = const.tile([128, NT], F32)

        cci = dram.tile([128, 8, 65], BF16)
        cco = dram.tile([128, 8, 65], BF16)
        z_d = dram.tile([NG, 2, 8, 512], BF16)
        h3s = dram.tile([32, 128, TOK], BF16)

        # --- persistent pools, ordered for LIFO-clean lifetimes ---
        # x_sb holds x through phases 1-2, then x1 (in place) until 3b
        xs_cm = tc.tile_pool(name="xsp", bufs=1)
        xsp = xs_cm.__enter__()
        x_sb = xsp.tile([128, NT, C], BF16)          # 32KB/p
        # qT holds phi(q)^T through ph2, then h2T (slot reuse) until 3a end
        qT_cm = tc.tile_pool(name="qTp", bufs=1)
        qTp = qT_cm.__enter__()
        qT = qTp.tile([128, 8, TOK], BF16)           # 32KB/p
        h2T = qT                                     # alias: groups reused
        # wp + kv2 enter early (fresh space -> DMAs overlap phase 1)
        wp_cm = tc.tile_pool(name="wpp", bufs=1)
        wpp = wp_cm.__enter__()
        wp_sb = wpp.tile([128, 8, C], BF16)
        nc.sync.dma_start(out=wp_sb[:], in_=wp.ap())
        kv2_cm = tc.tile_pool(name="kv2", bufs=1)
        kv2p = kv2_cm.__enter__()
        kv8 = kv2p.tile([128, 8, 65], BF16)
        kv_bd = kv2p.tile([128, 8, 128], BF16)
        bd = kv2p.tile([128, 8, 16], BF16)
        nc.vector.memset(kv_bd[:], 0.0)
        nc.vector.memset(bd[:], 0.0)

        # ---------------- Phase 1: LN1, hT, k/v/q, kv+ksum ----------------
        with (
            tc.tile_pool(name="wqkvp", bufs=1) as wqkvp,
            tc.tile_pool(name="hTg", bufs=2) as hTgp,
            tc.tile_pool(name="p1w", bufs=2) as p1w,
            tc.tile_pool(name="kvstage", bufs=1) as kvstagep,
            tc.tile_pool(name="kvacc_ps", bufs=1, space="PSUM") as kvaccp,
            tc.tile_pool(name="p1_ps", bufs=6, space="PSUM") as p1psp,
        ):
            # group-0 x tiles first so PE work can start before weights land
            def load_stats(tt):
                nc.sync.dma_start(out=x_sb[:, tt, :], in_=xs_v[tt])
                bnst = statp.tile([128, 2, 6], F32, tag="bnst")
                for sg in range(2):
                    nc.vector.bn_stats(out=bnst[:, sg, :],
                                       in_=x_sb[:, tt, sg * 512:(sg + 1) * 512])
                nc.vector.bn_aggr(out=mvall[:, tt, :], in_=bnst[:])

            for tt in range(4):
                load_stats(tt)
            wkv_sb = wqkvp.tile([128, 8, 2 * C], BF16)
            for oc in range(4):
                nc.sync.dma_start(out=wkv_sb[:, :, oc * 512:(oc + 1) * 512],
                                  in_=wkv.ap()[:, :, oc * 512:(oc + 1) * 512])
            wq_sb = wqkvp.tile([128, 8, C], BF16)
            for oc in range(2):
                nc.sync.dma_start(out=wq_sb[:, :, oc * 512:(oc + 1) * 512],
                                  in_=wq.ap()[:, :, oc * 512:(oc + 1) * 512])
            kv_ps = [kvaccp.tile([128, 4, 65], F32, name=f"kv_ps{i}") for i in range(2)]
            pending_kvacc = []

            def flush_kvacc():
                while pending_kvacc:
                    emit = pending_kvacc.pop(0)
                    emit()

            for g in range(NG):
                # stats for this group of 4 tiles
                for tl in range(4):
                    tt = 4 * g + tl
                    if tt >= 4:
                        load_stats(tt)
                g4 = 4 * g
                nc.scalar.activation(out=sdall[:, g4:g4 + 4], in_=mvall[:, g4:g4 + 4, 1],
                                     func=AF.Sqrt, bias=eps_ln_t[:], scale=1.0)
                nc.vector.reciprocal(out=rstdall[:, g4:g4 + 4], in_=sdall[:, g4:g4 + 4])
                nc.vector.scalar_tensor_tensor(
                    out=nmrall[:, g4:g4 + 4], in0=mvall[:, g4:g4 + 4, 0], scalar=-1.0,
                    in1=rstdall[:, g4:g4 + 4], op0=ALU.mult, op1=ALU.mult)

                hT_g = hTgp.tile([128, 8, 512], BF16, tag="hTg", name=f"hTg{g}")
                for tl in range(4):
                    tt = 4 * g + tl
                    h_t = p1w.tile([128, C], BF16, tag="h")
                    nc.vector.tensor_scalar(out=h_t[:], in0=x_sb[:, tt, :],
                                            scalar1=rstdall[:, tt:tt + 1],
                                            scalar2=nmrall[:, tt:tt + 1],
                                            op0=ALU.mult, op1=ALU.add)
                    # transpose h -> hT_g[:, :, tl*128:+128]
                    for half in range(2):
                        trp = p1psp.tile([128, 4, 128], BF16, tag="ps",
                                         name=f"tr{tt}_{half}")
                        for q in range(4):
                            cc = half * 4 + q
                            nc.tensor.transpose(trp[:, q, :],
                                                h_t[:, cc * 128:(cc + 1) * 128], ident[:])
                        nc.vector.tensor_copy(
                            out=hT_g[:, half * 4:(half + 1) * 4, tl * 128:(tl + 1) * 128],
                            in_=trp[:])
                    # k, v for this tile
                    k_sb = p1w.tile([128, C], BF16, tag="k")
                    v_ext = p1w.tile([128, H, 65], BF16, tag="v")
                    nc.vector.memset(v_ext[:, :, 64:65], 1.0)
                    ps4 = [p1psp.tile([128, 512], F32, tag="ps", name=f"gen{tt}_{i}")
                           for i in range(4)]
                    for cc in range(8):
                        for oc in range(4):
                            nc.tensor.matmul(ps4[oc][:],
                                             lhsT=hT_g[:, cc, tl * 128:(tl + 1) * 128],
                                             rhs=wkv_sb[:, cc, oc * 512:(oc + 1) * 512],
                                             start=(cc == 0), stop=(cc == 7))
                    flush_kvacc()   # prev tile's kv-acc: PE filler while phi runs
                    # phi(k) = exp(min(x,0)) + max(x,0); read each psum early
                    # (max+min first) so its slot frees before the exp chain.
                    mts, rts = [], []
                    for oc in range(2):
                        ps = ps4[oc]
                        if has_bk:
                            ksl = k_sb[:, oc * 512:(oc + 1) * 512]
                            nc.vector.tensor_tensor(
                                out=ksl, in0=ps[:],
                                in1=bk_bc[:, oc * 512:(oc + 1) * 512], op=ALU.add)
                            src = ksl
                        else:
                            src = ps[:]
                        mt = p1w.tile([128, 512], BF16, tag="phim",
                                      name=f"phim{tt}_{oc}")
                        rt = p1w.tile([128, 512], BF16, tag="phir",
                                      name=f"phir{tt}_{oc}")
                        nc.scalar.activation(out=rt[:], in_=src, func=AF.Relu)
                        # relu(-x) on ACT; exp(-relu(-x)) = exp(min(x,0)) later
                        nc.scalar.activation(out=mt[:], in_=src, func=AF.Relu,
                                             scale=-1.0)
                        mts.append(mt)
                        rts.append(rt)
                    for oc in range(4):
                        ps = ps4[oc]
                        if oc < 2:
                            mt, rt = mts[oc], rts[oc]
                            ksl = k_sb[:, oc * 512:(oc + 1) * 512]
                            nc.scalar.activation(out=mt[:], in_=mt[:], func=AF.Exp,
                                                 scale=-1.0)
                            nc.vector.tensor_tensor(out=ksl, in0=mt[:], in1=rt[:],
                                                    op=ALU.add)
                        else:      # v -> v_ext[:, heads, 0:64]
                            h0 = (oc - 2) * 8
                            dst = v_ext[:, h0:h0 + 8, 0:64]
                            if has_bv:
                                vb = bass.AP(tensor=bv.ap().tensor, offset=(oc - 2) * 512,
                                             ap=[[0, 128], [64, 8], [1, 64]])
                                vb_t = p1w.tile([128, 8, 64], F32, tag="vb")
                                nc.sync.dma_start(out=vb_t[:], in_=vb)
                                nc.vector.tensor_tensor(
                                    out=dst, in0=ps[:].rearrange("p (h d) -> p h d", d=64),
                                    in1=vb_t[:], op=ALU.add)
                            else:
                                nc.scalar.activation(
                                    out=dst, in_=ps[:].rearrange("p (h d) -> p h d", d=64),
                                    func=AF.Identity)
                    # kv accumulation: per head [64, 65] += k_h^T @ [v_h | 1]
                    # (pipelined: emitted at the start of the NEXT tile so the
                    # PE has work while phi(k) runs on DVE/ACT)
                    def emit_kvacc(tt=tt, k_sb=k_sb, v_ext=v_ext):
                        for h in range(H):
                            ti, hf, slot = h // 8, (h % 8) // 4, h % 4
                            nc.tensor.matmul(
                                kv_ps[ti][hf * 64:(hf + 1) * 64, slot, :],
                                lhsT=k_sb[:, h * 64:(h + 1) * 64],
                                rhs=v_ext[:, h, :],
                                start=(tt == 0), stop=(tt == NT - 1))
                    pending_kvacc.append(emit_kvacc)

                if g == NG - 1:
                    flush_kvacc()
                    # stage kv psum -> SBUF -> DRAM -> pairwise AllReduce,
                    # overlapping the last group's q computation below.
                    # cci layout [128, 8, 65]: head h at partitions (h%2)*64,
                    # column h//2 — matches the kv8 layout used by phase 2.
                    kv_st = kvstagep.tile([128, 8, 65], BF16)
                    for h in range(H):
                        ti, hf, slot = h // 8, (h % 8) // 4, h % 4
                        pbase = (h % 2) * 64
                        nc.vector.tensor_copy(
                            out=kv_st[pbase:pbase + 64, h // 2, :],
                            in_=kv_ps[ti][hf * 64:(hf + 1) * 64, slot, :])
                    nc.scalar.dma_start(out=cci[:], in_=kv_st[:])
                    if no_cc:
                        nc.scalar.dma_start(out=cco[:], in_=cci[:])
                    else:
                        nc.gpsimd.collective_compute(
                            "AllReduce", ALU.add,
                            replica_groups=[[0, 1], [2, 3], [4, 5], [6, 7]],
                            ins=[cci[:]], outs=[cco[:]])
                    # rebuild block-diag kv + ksum tiles on DVE (runs as cco lands)
                    nc.scalar.dma_start(out=kv8[:], in_=cco[:])
                    for h in range(H):
                        pbase = (h % 2) * 64
                        r = 8 * (h % 2) + h // 2
                        nc.vector.tensor_copy(
                            out=kv_bd[pbase:pbase + 64, h // 2, pbase:pbase + 64],
                            in_=kv8[pbase:pbase + 64, h // 2, 0:64])
                        nc.vector.tensor_copy(
                            out=bd[pbase:pbase + 64, h // 2, r:r + 1],
                            in_=kv8[pbase:pbase + 64, h // 2, 64:65])

                # q for this group -> qT (phi applied)
                for oc in range(8):
                    ps = p1psp.tile([128, 512], F32, tag="ps", name=f"q{g}_{oc}")
                    for cc in range(8):
                        nc.tensor.matmul(ps[:], lhsT=wq_sb[:, cc, oc * 128:(oc + 1) * 128],
                                         rhs=hT_g[:, cc, :],
                                         start=(cc == 0), stop=(cc == 7))
                    mt = p1w.tile([128, 512], BF16, tag="phim")
                    rt = p1w.tile([128, 512], BF16, tag="phir")
                    nc.vector.tensor_scalar(out=mt[:], in0=ps[:], scalar1=bq_sb[:, oc:oc + 1],
                                            scalar2=0.0, op0=ALU.add, op1=ALU.min)
                    nc.scalar.activation(out=rt[:], in_=ps[:], func=AF.Relu,
                                         bias=bq_sb[:, oc:oc + 1], scale=1.0)
                    nc.scalar.activation(out=mt[:], in_=mt[:], func=AF.Exp)
                    nc.vector.tensor_tensor(out=qT[:, oc, g * 512:(g + 1) * 512],
                                            in0=mt[:], in1=rt[:], op=ALU.add)

        # w1/gelu pools enter before phase 2: their space reuses phase-1
        # pools (WAR clears at phase-1 end), so fc1 prefetch runs early.
        w1_cm = tc.tile_pool(name="w1p", bufs=3)
        w1p = w1_cm.__enter__()
        gel_cm = tc.tile_pool(name="gelt", bufs=2)
        geltp = gel_cm.__enter__()

        # ---------------- Phase 2: attention + proj + LN2 ----------------
        # psum pool order matters: the stack allocator reuses banks bottom-up,
        # so z/attn (idle after the attention section) go first — fc1's psums
        # then land on banks whose last access is early, not proj's.
        with (
            tc.tile_pool(name="p2w", bufs=3) as p2w,
            tc.tile_pool(name="attnt", bufs=2) as attntp,
            tc.tile_pool(name="zbcpa", bufs=3) as zbcpa,
            tc.tile_pool(name="z_ps", bufs=1, space="PSUM") as zpsp,
            tc.tile_pool(name="attn_ps", bufs=2, space="PSUM") as attnpsp,
            tc.tile_pool(name="proj_ps", bufs=2, space="PSUM") as projpsp,
            tc.tile_pool(name="tr2_ps", bufs=1, space="PSUM") as trps2p,
        ):
            z_bcs = {}

            def emit_z(g):
                # z = 1 / (q . ksum + eps); bd maps head h -> psum row
                # 8*(h%2) + h//2, so rows 0-7 are even heads, 8-15 odd.
                zps = zpsp.tile([16, 512], F32, name=f"zps{g}", tag="zps")
                for pc in range(8):
                    nc.tensor.matmul(zps[:], lhsT=bd[:, pc, :],
                                     rhs=qT[:, pc, g * 512:(g + 1) * 512],
                                     start=(pc == 0), stop=(pc == 7))
                zslf = p2w.tile([16, 512], F32, name=f"ztf{g}", tag="ztf")
                nc.vector.tensor_scalar_add(out=zslf[:], in0=zps[:], scalar1=EPS_ATTN)
                zsl = p2w.tile([16, 512], BF16, name=f"zt{g}", tag="zt")
                with nc.allow_low_precision(reason="z factor tolerates bf16"):
                    nc.vector.reciprocal(out=zsl[:], in_=zslf[:])
                for sub in range(2):
                    nc.scalar.dma_start(out=z_d[g, sub],
                                        in_=zsl[sub * 8:(sub + 1) * 8, :])
                z_bc = zbcpa.tile([128, 8, 512], BF16, name=f"zbc{g}", tag="zbc")
                zd_ap = z_d[:]
                for sub in range(2):
                    nc.scalar.dma_start(
                        out=z_bc[sub * 64:(sub + 1) * 64, :, :],
                        in_=bass.AP(tensor=zd_ap.tensor,
                                    offset=zd_ap.offset + (g * 2 + sub) * 4096,
                                    ap=[[0, 64], [1, 4096]]))
                z_bcs[g] = z_bc

            emit_z(0)
            emit_z(1)
            for g in range(NG):
                if g + 2 < NG:
                    emit_z(g + 2)
                z_bc = z_bcs.pop(g)
                attn_r = attntp.tile([128, 8, 512], BF16, tag="attnr", name=f"attnr{g}")
                for cc in range(8):
                    aps = attnpsp.tile([128, 512], F32, tag="aps")
                    nc.tensor.matmul(aps[:], lhsT=kv_bd[:, cc, :],
                                     rhs=qT[:, cc, g * 512:(g + 1) * 512],
                                     start=True, stop=True)
                    nc.vector.tensor_tensor(out=attn_r[:, cc, :], in0=aps[:],
                                            in1=z_bc[:, cc, :], op=ALU.mult)

                for tl in range(4):
                    tt = g * 4 + tl
                    pps = projpsp.tile([128, C], F32, tag="pps")
                    for oc in range(2):
                        for cc in range(8):
                            nc.tensor.matmul(pps[:, oc * 512:(oc + 1) * 512],
                                             lhsT=attn_r[:, cc, tl * 128:(tl + 1) * 128],
                                             rhs=wp_sb[:, cc, oc * 512:(oc + 1) * 512],
                                             start=(cc == 0), stop=(cc == 7))
                    # x1 = proj + x (in place over x_sb); accumulate sum(x1)
                    # for LN2 in the same op. Second op squares for sum(x1^2).
                    x1sl = x_sb[:, tt, :]
                    mv2 = statp.tile([128, 4], F32, tag="mv2")
                    if has_bproj:
                        nc.vector.tensor_tensor(out=x1sl, in0=pps[:], in1=bp_bc[:],
                                                op=ALU.add)
                        nc.vector.scalar_tensor_tensor(
                            out=x1sl, in0=x1sl, scalar=0.0, in1=x_sb[:, tt, :],
                            op0=ALU.add, op1=ALU.add, accum_out=mv2[:, 0:1])
                    else:
                        nc.vector.scalar_tensor_tensor(
                            out=x1sl, in0=pps[:], scalar=0.0, in1=x_sb[:, tt, :],
                            op0=ALU.add, op1=ALU.add, accum_out=mv2[:, 0:1])
                    sqt = p2w.tile([128, C], BF16, tag="sqt")
                    nc.vector.scalar_tensor_tensor(
                        out=sqt[:], in0=x1sl, scalar=0.0, in1=x1sl,
                        op0=ALU.add, op1=ALU.mult, accum_out=mv2[:, 1:2])
                    # mu = s1/C ; var = s2/C - mu^2 ; rstd = 1/sqrt(var+eps)
                    nc.vector.tensor_scalar_mul(out=mv2[:, 0:1], in0=mv2[:, 0:1],
                                                scalar1=1.0 / C)
                    nc.vector.tensor_tensor(out=mv2[:, 2:3], in0=mv2[:, 0:1],
                                            in1=mv2[:, 0:1], op=ALU.mult)
                    nc.vector.scalar_tensor_tensor(
                        out=mv2[:, 1:2], in0=mv2[:, 1:2], scalar=1.0 / C,
                        in1=mv2[:, 2:3], op0=ALU.mult, op1=ALU.subtract)
                    nc.scalar.activation(out=mv2[:, 1:2], in_=mv2[:, 1:2],
                                         func=AF.Sqrt, bias=eps_ln_t[:], scale=1.0)
                    nc.vector.reciprocal(out=mv2[:, 1:2], in_=mv2[:, 1:2])
                    nc.vector.scalar_tensor_tensor(
                        out=mv2[:, 0:1], in0=mv2[:, 0:1], scalar=-1.0,
                        in1=mv2[:, 1:2], op0=ALU.mult, op1=ALU.mult)
                    h2 = p2w.tile([128, C], BF16, tag="h2")
                    nc.vector.tensor_scalar(out=h2[:], in0=x1sl,
                                            scalar1=mv2[:, 1:2], scalar2=mv2[:, 0:1],
                                            op0=ALU.mult, op1=ALU.add)
                    # h2T overwrites qT[:, :, g*512...] — qT(g) fully consumed
                    for half in range(2):
                        trp2 = trps2p.tile([128, 4, 128], BF16, tag="tr2")
                        for q in range(4):
                            cc = half * 4 + q
                            nc.tensor.transpose(trp2[:, q, :],
                                                h2[:, cc * 128:(cc + 1) * 128], ident[:])
                        nc.vector.tensor_copy(
                            out=h2T[:, half * 4:(half + 1) * 4, tt * 128:(tt + 1) * 128],
                            in_=trp2[:])
        # ---------------- Phase 3a: fc1 + gelu -> h3s ----------------
        with tc.tile_pool(name="f1_ps", bufs=6, space="PSUM") as f1psp:
            for hd in range(32):
                w1c = w1p.tile([128, 8, 128], BF16)
                # SWDGE (Pool) ring: idle engine, dodges SP queue head-of-line
                nc.gpsimd.dma_start(out=w1c[:], in_=w1.ap()[hd])
                gl = geltp.tile([128, TOK], BF16)
                for gg in range(4):
                    ps = f1psp.tile([128, 512], F32, tag="f1", name=f"f1_{hd}_{gg}")
                    for cc in range(8):
                        nc.tensor.matmul(
                            ps[:], lhsT=w1c[:, cc, :],
                            rhs=h2T[:, cc, gg * 512:(gg + 1) * 512],
                            start=(cc == 0), stop=(cc == 7))
                    nc.scalar.activation(out=gl[:, gg * 512:(gg + 1) * 512],
                                         in_=ps[:], func=AF.Gelu,
                                         bias=bg_sb[:, hd:hd + 1], scale=1.0)
                nc.scalar.dma_start(out=h3s[hd], in_=gl[:])
        # load w2[hd 16:31] into the (now dead) qT/h2T tile: same 16K elems/p.
        # fc2 rhs slice for hd, oc lives at qT[:, hd//2, (hd%2)*1024 + oc*512].
        hb = H  # 16
        nc.sync.dma_start(
            out=qT[:],
            in_=w2.ap()[:, 16:32, :].rearrange("p h c -> p (h c)").rearrange(
                "p (a b) -> p a b", a=8))
        gel_cm.__exit__(None, None, None)
        w1_cm.__exit__(None, None, None)
        kv2_cm.__exit__(None, None, None)
        wp_cm.__exit__(None, None, None)

        # ---------------- Phase 3b: fc2 (streamed w2) + residual -> out ----------------
        with tc.tile_pool(name="h3c", bufs=2) as h3cp, \
             tc.tile_pool(name="w2c", bufs=3) as w2cp, \
             tc.tile_pool(name="outp", bufs=2) as outp, \
             tc.tile_pool(name="f2_ps", bufs=4, space="PSUM") as f2psp:
            h3s_v = h3s[:].rearrange("hd p t -> p hd t")

            def w2b_slice(hd, oc):
                hl = hd - 16
                base = (hl % 2) * 1024 + oc * 512
                return qT[:, hl // 2, base:base + 512]

            for q4 in range(4):
                h3c = h3cp.tile([128, 32, 512], BF16, tag="h3c", name=f"h3c{q4}")
                # split by hd-halves: the first half's h3s rows finish at 3a's
                # midpoint, so quad 0 can load early
                for hh in range(2):
                    # SWDGE: separate completion lanes — avoids inheriting the
                    # whole HWDGE drain (h3s tail writes) via lane counters
                    nc.gpsimd.dma_start(
                        out=h3c[:, 16 * hh:16 * (hh + 1), :],
                        in_=h3s_v[:, 16 * hh:16 * (hh + 1),
                                  q4 * 512:(q4 + 1) * 512])
                pst4 = [f2psp.tile([128, C], F32, tag="pst", name=f"pst{q4}_{tl}")
                        for tl in range(4)]
                for hq in range(4):     # hd 0..15 streamed
                    w2c = w2cp.tile([128, 4, C], BF16, tag="w2c")
                    nc.gpsimd.dma_start(out=w2c[:],
                                        in_=w2.ap()[:, 4 * hq:4 * (hq + 1), :])
                    for hi in range(4):
                        hd = 4 * hq + hi
                        for tl in range(4):
                            for oc in range(2):
                                nc.tensor.matmul(
                                    pst4[tl][:, oc * 512:(oc + 1) * 512],
                                    lhsT=h3c[:, hd, tl * 128:(tl + 1) * 128],
                                    rhs=w2c[:, hi, oc * 512:(oc + 1) * 512],
                                    start=(hd == 0), stop=(hd == 31))
                # hd 16..31 from the qT-resident half, per tile so each psum
                # evacuates while the next tile's matmuls run
                for tl in range(4):
                    tt = q4 * 4 + tl
                    for hd in range(16, 32):
                        for oc in range(2):
                            nc.tensor.matmul(
                                pst4[tl][:, oc * 512:(oc + 1) * 512],
                                lhsT=h3c[:, hd, tl * 128:(tl + 1) * 128],
                                rhs=w2b_slice(hd, oc),
                                start=False, stop=(hd == 31))
                    o_t = outp.tile([128, C], F32, tag="ot")
                    nc.vector.tensor_tensor(out=o_t[:], in0=pst4[tl][:],
                                            in1=x_sb[:, tt, :], op=ALU.add)
                    if has_bfc2:
                        nc.vector.tensor_tensor(out=o_t[:], in0=o_t[:],
                                                in1=b2_bc[:], op=ALU.add)
                    nc.scalar.dma_start(out=out_v[tt], in_=o_t[:])
        qT_cm.__exit__(None, None, None)
        xs_cm.__exit__(None, None, None)

    nc.compile()
    return nc


def _prep_inputs(x, norm1_g, norm1_b, qkv_w, proj_w, proj_b, norm2_g, norm2_b,
                 fc1_w, fc1_b, fc2_w, fc2_b):
    """Host-side weight prep. Folds LN gains into weights; LN biases into
    per-output biases. Weights pre-swizzled to SBUF layouts, cast to bf16."""
    BF = ml_dtypes.bfloat16
    x = np.asarray(x, np.float32)
    g1 = np.asarray(norm1_g, np.float32)
    b1 = np.asarray(norm1_b, np.float32)
    qkv_w = np.asarray(qkv_w, np.float32)
    proj_w = np.asarray(proj_w, np.float32)
    proj_b = np.asarray(proj_b, np.float32)
    g2 = np.asarray(norm2_g, np.float32)
    b2 = np.asarray(norm2_b, np.float32)
    fc1_w = np.asarray(fc1_w, np.float32)
    fc1_b = np.asarray(fc1_b, np.float32)
    fc2_w = np.asarray(fc2_w, np.float32)
    fc2_b = np.asarray(fc2_b, np.float32)

    wq_t = (qkv_w[0:C] * g1[None, :]).T                      # [c, o]
    wkv_t = (qkv_w[C:3 * C] * g1[None, :]).T                 # [c, 2C]
    wp_t = proj_w.T
    w1_t = (fc1_w * g2[None, :]).T                           # [c, HID]
    w2_t = fc2_w.T                                           # [HID, c]

    wq_l = np.ascontiguousarray(
        wq_t.reshape(8, 128, C).transpose(1, 0, 2)).astype(BF)
    wkv_l = np.ascontiguousarray(
        wkv_t.reshape(8, 128, 2 * C).transpose(1, 0, 2)).astype(BF)
    wp_l = np.ascontiguousarray(
        wp_t.reshape(8, 128, C).transpose(1, 0, 2)).astype(BF)
    w1_l = np.ascontiguousarray(
        w1_t.reshape(8, 128, 32, 128).transpose(2, 1, 0, 3)).astype(BF)
    w2_l = np.ascontiguousarray(
        w2_t.reshape(32, 128, C).transpose(1, 0, 2)).astype(BF)

    bq_v = qkv_w[0:C] @ b1
    bk_v = qkv_w[C:2 * C] @ b1
    bv_v = qkv_w[2 * C:3 * C] @ b1
    bg_v = fc1_w @ b2 + fc1_b

    flags = (bool(np.any(bk_v)), bool(np.any(bv_v)),
             bool(np.any(proj_b)), bool(np.any(fc2_b)))

    shared = dict(wq=wq_l, wkv=wkv_l, wp=wp_l, w1=w1_l, w2=w2_l,
                  bq=np.ascontiguousarray(bq_v, dtype=np.float32),
                  bk=np.ascontiguousarray(bk_v, dtype=np.float32),
                  bv=np.ascontiguousarray(bv_v, dtype=np.float32),
                  bg=np.ascontiguousarray(bg_v, dtype=np.float32),
                  bp=proj_b, b2o=fc2_b)
    in_maps = []
    for core in range(8):
        b, half = core // 2, core % 2
        xs = np.ascontiguousarray(x[b, half * TOK:(half + 1) * TOK, :]).astype(BF)
        in_maps.append({"xs": xs, **shared})
    return flags, in_maps


def get_compiled(flags):
    if flags not in _BUILD_CACHE:
        _BUILD_CACHE[flags] = _build(flags)
    return _BUILD_CACHE[flags]


def kernel(**inputs) -> np.ndarray:
    flags, in_maps = _prep_inputs(**inputs)
    nc = get_compiled(flags)
    res = run_bass_kernel_spmd(nc, in_maps=in_maps, core_ids=list(range(8)))
    shards = [res.results[c]["out"] for c in range(8)]
    full = np.empty((B, N, C), np.float32)
    for core in range(8):
        b, half = core // 2, core % 2
        full[b, half * TOK:(half + 1) * TOK, :] = shards[core]
    return full
